# revision 20
# baseline (speedup 1.0000x reference)
"""ComplexGRUCell forward on 8 Trainium2 NeuronCores.

v4 strategy (data-parallel, feature-major compute):
  - Shard batch B=65536 across 8 cores (8192 rows each).
  - 34 matmul passes per (tile, mo) [was 36]:
      r gate:  fp8 DoubleRow, contraction 1024, re+im      -> 8 passes
      z gate:  x-part fp8 DR (4), hr-part fp8 DR (2),
               hi-part fp16 (4)                            -> 10 passes
      x3:      fp16, contraction 512, re+im                -> 8 passes
      g3:      fp16, contraction 512, re+im                -> 8 passes
    Mix chosen by offline precision sim: L2 rel ~1.85e-2 < 2e-2 gate.
  - Sign-folded epilogue: the im-halves of x3/g3/out carry a factor -1
    (folded into host-packed weights/biases and un-done on host gather),
    which makes every wide element-wise op a uniform add/sub/mul over
    flat [128, 1024] APs (3D APs halve DVE throughput).
  - Engine balance: Scalar 8 ACTs; DVE wide muls/adds; GpSimd the 4
    cross-term muls of the two complex products.
  - z-gate matmuls mid-scheduled so sigmoid(z) fires before x3 finishes,
    shortening the last-tile drain.

Self-contained: hardcodes B=65536, I=H=256, 8 cores.
"""

import numpy as np
import ml_dtypes

import concourse.bass as bass  # noqa: F401
import concourse.mybir as mybir
import concourse.tile as tile
from concourse import bacc, bass_utils
from concourse.alu_op_type import AluOpType as OP

F32 = mybir.dt.float32
F16 = mybir.dt.float16
F8 = mybir.dt.float8e4
AF = mybir.ActivationFunctionType
PM = mybir.MatmulPerfMode

B_TOTAL = 65536
N_CORES = 8
B_LOC = B_TOTAL // N_CORES  # 8192
H = 256
NB = 512                    # batch columns per tile
N_TILES = B_LOC // NB       # 16

S_MOV = 16.0                # fp8 moving-data scale
S_WGT = 256.0               # fp8 weight scale
SC8 = S_MOV * S_WGT         # fp8-psum scale (4096)
INV_S = 1.0 / SC8
E4M3 = ml_dtypes.float8_e4m3

# bias columns: 8 logical biases x 2 mo
_BIAS_NAMES = ['r_re', 'r_im', 'z_re', 'z_im',
               'g3n_re', 'g3n_im', 'x3n_re', 'x3n_im']

# Module-level knobs for the test harness (grading path leaves them alone).
TRACE = False
LAST_RESULT = None

_CACHED_NC = None


def _build_nc():
    nc = bacc.Bacc("TRN2", target_bir_lowering=False, debug=False,
                   num_devices=N_CORES)

    ins = {}
    # fp8 streams, tile-major: [128, tile, chunk(8), NB]
    # chunk order: xr0 xr1 xi0 xi1 hr0 hr1 hi0 hi1  (DR pairs (0,1)(2,3)..)
    ins["s8"] = nc.dram_tensor("s8", (128, N_TILES * 8 * NB), F8,
                               kind="ExternalInput")
    # fp16 streams, tile-major: [128, tile, chunk(8), NB]
    # chunk order: hr0 nhi0 hr1 nhi1 xr0 xr1 xi0 xi1   (nhi = -hi)
    ins["s16"] = nc.dram_tensor("s16", (128, N_TILES * 8 * NB), F16,
                                kind="ExternalInput")
    # fp8 DR stationary: rows pair up ((2k,2k+1) = one DR pass)
    # r: rows 0:32, zx: 32:48, zh-hr: 48:56
    ins["w8"] = nc.dram_tensor("w8", (128, 56, 128), F8,
                               kind="ExternalInput")
    # fp16 stationary rows: g3: 0:16, zh-hi: 16:24, x3: 24:40
    ins["w16"] = nc.dram_tensor("w16", (128, 40, 128), F16,
                                kind="ExternalInput")
    ins["biases"] = nc.dram_tensor("biases", (128, 16), F32,
                                   kind="ExternalInput")
    # output: [feature, re/nim, batch]  (nim = -im, host negates)
    outT = nc.dram_tensor("outT", (H, 2, B_LOC), F16, kind="ExternalOutput")

    bias_col = {}
    for gi, g in enumerate(_BIAS_NAMES):
        for mo in range(2):
            bias_col[(g, mo)] = gi * 2 + mo

    with tile.TileContext(nc) as tc:
        with (
            tc.tile_pool(name="wpool", bufs=1) as wpool,
            tc.tile_pool(name="m8pool", bufs=3) as m8pool,
            tc.tile_pool(name="m16pool", bufs=3) as m16pool,
            tc.tile_pool(name="spool", bufs=2) as spool,
            tc.tile_pool(name="tpool", bufs=2) as tpool,
            tc.tile_pool(name="opool", bufs=3) as opool,
            tc.tile_pool(name="psum", bufs=1, space="PSUM") as psum,
        ):
            # ---- loads: moving tile 0 first, then weights ----------------
            def load_m8(t_idx):
                t0 = t_idx * (8 * NB)
                t = m8pool.tile([128, 8, NB], F8, name="m8", tag="m8")
                nc.sync.dma_start(t[:], ins["s8"][:, t0:t0 + 8 * NB])
                return t

            def load_m16(t_idx):
                t0 = t_idx * (8 * NB)
                t = m16pool.tile([128, 8 * NB], F16, name="m16", tag="m16")
                nc.sync.dma_start(t[:], ins["s16"][:, t0:t0 + 8 * NB])
                return t

            # All prologue DMAs on the fast sync queue, in just-in-time
            # order: biases, r-weights, m8 t0, zx/zh weights, h-half of
            # m16 t0, g3+zh fp16 weights, x-half of m16 t0, x3 weights.
            w8t = wpool.tile([128, 56, 128], F8, name="w8t", tag="w8t")
            nc.sync.dma_start(w8t[:, 0:32, :], ins["w8"][:, 0:32, :])
            m8_0 = load_m8(0)
            bt = wpool.tile([128, 16], F32, name="bias_t", tag="bias_t")
            nc.sync.dma_start(bt[:], ins["biases"][:])
            nc.sync.dma_start(w8t[:, 32:56, :], ins["w8"][:, 32:56, :])
            m16_0 = m16pool.tile([128, 8 * NB], F16, name="m16", tag="m16")
            nc.sync.dma_start(m16_0[:, 0:4 * NB], ins["s16"][:, 0:4 * NB])
            w16t = wpool.tile([128, 40, 128], F16, name="w16t", tag="w16t")
            nc.sync.dma_start(w16t[:, 0:24, :], ins["w16"][:, 0:24, :])
            nc.sync.dma_start(m16_0[:, 4 * NB:8 * NB], ins["s16"][:, 4 * NB:8 * NB])
            nc.sync.dma_start(w16t[:, 24:40, :], ins["w16"][:, 24:40, :])

            def bias_ap(g, mo):
                c = bias_col[(g, mo)]
                return bt[:, c:c + 1]

            # ---- per batch tile ------------------------------------------
            for t_idx in range(N_TILES):
                if t_idx == 0:
                    m8, m16 = m8_0, m16_0
                else:
                    m8 = load_m8(t_idx)
                    m16 = load_m16(t_idx)

                for mo in range(2):
                    p_r = psum.tile([128, 2 * NB], F32, name=f"pr{mo}",
                                    tag="bkA")
                    p_z = psum.tile([128, 2 * NB], F32, name=f"pz{mo}",
                                    tag="bkB")
                    p_g3 = psum.tile([128, 2 * NB], F32, name=f"pg{mo}",
                                     tag="bkC")
                    p_x3 = psum.tile([128, 2 * NB], F32, name=f"px{mo}",
                                     tag="bkD")

                    # r gate: pairs ((comp*2+mo)*4 + si), si = 4 streams
                    for comp in range(2):
                        dst = p_r[:, comp * NB:(comp + 1) * NB]
                        for si in range(4):
                            k = ((comp * 2 + mo) * 4 + si) * 2
                            nc.tensor.matmul(
                                dst, w8t[:, k:k + 2, :],
                                m8[:, 2 * si:2 * si + 2, :],
                                start=(si == 0), stop=(si == 3),
                                perf_mode=PM.DoubleRow)
                    # zx: pairs 16 + (comp*2+mo)*2 + si, si over xr, xi
                    for comp in range(2):
                        dst = p_z[:, comp * NB:(comp + 1) * NB]
                        for si in range(2):
                            k = (16 + (comp * 2 + mo) * 2 + si) * 2
                            nc.tensor.matmul(
                                dst, w8t[:, k:k + 2, :],
                                m8[:, 2 * si:2 * si + 2, :],
                                start=(si == 0), stop=False,
                                perf_mode=PM.DoubleRow)
                    # zh-hr: pair 24 + comp*2+mo, moving = (hr0, hr1)
                    for comp in range(2):
                        dst = p_z[:, comp * NB:(comp + 1) * NB]
                        k = (24 + comp * 2 + mo) * 2
                        nc.tensor.matmul(
                            dst, w8t[:, k:k + 2, :], m8[:, 4:6, :],
                            start=False, stop=False,
                            perf_mode=PM.DoubleRow)
                    # g3: rows (comp*2+mo)*4 + j, moving chunks j=0..3
                    for comp in range(2):
                        dst = p_g3[:, comp * NB:(comp + 1) * NB]
                        for j in range(4):
                            row = (comp * 2 + mo) * 4 + j
                            nc.tensor.matmul(
                                dst, w16t[:, row, :],
                                m16[:, j * NB:(j + 1) * NB],
                                start=(j == 0), stop=(j == 3))
                    # zh-hi: rows 16 + (comp*2+mo)*2 + j, moving nhi0/1
                    for comp in range(2):
                        dst = p_z[:, comp * NB:(comp + 1) * NB]
                        for j in range(2):
                            row = 16 + (comp * 2 + mo) * 2 + j
                            nc.tensor.matmul(
                                dst, w16t[:, row, :],
                                m16[:, (1 + 2 * j) * NB:(2 + 2 * j) * NB],
                                start=False, stop=(j == 1))
                    # x3: rows 24 + (comp*2+mo)*4 + j, moving chunks 4..7
                    for comp in range(2):
                        dst = p_x3[:, comp * NB:(comp + 1) * NB]
                        for j in range(4):
                            row = 24 + (comp * 2 + mo) * 4 + j
                            nc.tensor.matmul(
                                dst, w16t[:, row, :],
                                m16[:, (4 + j) * NB:(5 + j) * NB],
                                start=(j == 0), stop=(j == 3))

                    # --- epilogue ------------------------------------------
                    sr = spool.tile([128, 2 * NB], F16, name=f"sr{mo}",
                                    tag="sr")
                    gp = spool.tile([128, 2 * NB], F16, name=f"gp{mo}",
                                    tag="gp")
                    sz = spool.tile([128, 2 * NB], F16, name=f"sz{mo}",
                                    tag="sz")
                    P = tpool.tile([128, 2 * NB], F16, name=f"P{mo}", tag="P")
                    qh = tpool.tile([128, NB], F16, name=f"qh{mo}", tag="qh")
                    ql = tpool.tile([128, NB], F16, name=f"ql{mo}", tag="ql")
                    h3 = tpool.tile([128, 2 * NB], F16, name=f"h3{mo}",
                                    tag="h3")
                    ss = tpool.tile([128, 2 * NB], F16, name=f"ss{mo}",
                                    tag="ss")
                    nn = spool.tile([128, 2 * NB], F16, name=f"nn{mo}",
                                    tag="nn")
                    d = tpool.tile([128, 2 * NB], F16, name=f"d{mo}", tag="d")
                    P2 = tpool.tile([128, 2 * NB], F16, name=f"P2{mo}",
                                    tag="P2")
                    q2h = tpool.tile([128, NB], F16, name=f"q2h{mo}",
                                     tag="q2h")
                    q2l = tpool.tile([128, NB], F16, name=f"q2l{mo}",
                                     tag="q2l")
                    tm = tpool.tile([128, 2 * NB], F16, name=f"tm{mo}",
                                    tag="tm")
                    ot = opool.tile([128, 2 * NB], F16, name=f"ot{mo}",
                                    tag="ot")

                    nc.scalar.activation(sr[:, 0:NB], p_r[:, 0:NB],
                                         AF.Sigmoid, bias=bias_ap("r_re", mo),
                                         scale=INV_S)
                    nc.scalar.activation(sr[:, NB:], p_r[:, NB:],
                                         AF.Sigmoid, bias=bias_ap("r_im", mo),
                                         scale=INV_S)
                    # G' = [g3_re + b | -(g3_im + b)] (signs in wts/bias)
                    nc.scalar.activation(gp[:, 0:NB], p_g3[:, 0:NB],
                                         AF.Identity,
                                         bias=bias_ap("g3n_re", mo))
                    nc.scalar.activation(gp[:, NB:], p_g3[:, NB:],
                                         AF.Identity,
                                         bias=bias_ap("g3n_im", mo))
                    # h3 = [h3_re | -h3_im]
                    nc.vector.tensor_mul(P[:], sr[:], gp[:])
                    nc.vector.tensor_mul(qh[:], sr[:, 0:NB], gp[:, NB:])
                    nc.vector.tensor_mul(ql[:], sr[:, NB:], gp[:, 0:NB])
                    nc.vector.tensor_add(h3[:, 0:NB], P[:, 0:NB], P[:, NB:])
                    nc.vector.tensor_sub(h3[:, NB:], qh[:], ql[:])
                    # ss = p_x3' + h3   (both carry -im)
                    nc.vector.tensor_add(ss[:], p_x3[:], h3[:])
                    nc.scalar.activation(nn[:, 0:NB], ss[:, 0:NB], AF.Tanh,
                                         bias=bias_ap("x3n_re", mo))
                    nc.scalar.activation(nn[:, NB:], ss[:, NB:], AF.Tanh,
                                         bias=bias_ap("x3n_im", mo))
                    # d' = h' - nn' = [d_re | -d_im]
                    hflat = m16[:, 2 * mo * NB:(2 * mo + 2) * NB]
                    nc.vector.tensor_sub(d[:], hflat, nn[:])
                    nc.scalar.activation(sz[:, 0:NB], p_z[:, 0:NB],
                                         AF.Sigmoid, bias=bias_ap("z_re", mo),
                                         scale=INV_S)
                    nc.scalar.activation(sz[:, NB:], p_z[:, NB:],
                                         AF.Sigmoid, bias=bias_ap("z_im", mo),
                                         scale=INV_S)
                    # t' = [t_re | -t_im] = z (x) d
                    nc.vector.tensor_mul(P2[:], sz[:], d[:])
                    nc.vector.tensor_mul(q2h[:], sz[:, 0:NB], d[:, NB:])
                    nc.vector.tensor_mul(q2l[:], sz[:, NB:], d[:, 0:NB])
                    nc.vector.tensor_add(tm[:, 0:NB], P2[:, 0:NB],
                                         P2[:, NB:])
                    nc.vector.tensor_sub(tm[:, NB:], q2h[:], q2l[:])
                    # ot' = nn' + t' = [out_re | -out_im]
                    nc.vector.tensor_add(ot[:], nn[:], tm[:])
                    c0 = t_idx * NB
                    ot3 = ot[:].rearrange("p (c n) -> p c n", c=2)
                    nc.sync.dma_start(
                        outT[mo * 128:(mo + 1) * 128, :, c0:c0 + NB],
                        ot3[:])

    nc.compile()
    return nc


def _prep_weights(p):
    """Pack stationary weights + biases (host side, layout only)."""
    def T(name):
        return np.asarray(p[name], np.float32).T  # [in, out]

    w1r, w1i = T('w1Wr'), T('w1Wi')
    r1r, r1i = T('r1Wr'), T('r1Wi')
    w2r, w2i = T('w2Wr'), T('w2Wi')
    r2r, r2i = T('r2Wr'), T('r2Wi')
    w3r, w3i = T('w3Wr'), T('w3Wi')
    r3r, r3i = T('r3Wr'), T('r3Wi')

    # --- fp8 stationary ------------------------------------------------
    w8 = np.zeros((128, 56, 128), dtype=np.float32)

    def put8(row, block):
        w8[:, row, :] = block

    # r gate: stacked contraction [xr xi hr hi] (1024)
    r_stat = {
        0: np.concatenate([w1r, -w1i, r1r, -r1i], axis=0),   # re
        1: np.concatenate([w1i, w1r, r1i, r1r], axis=0),     # im
    }
    for comp in range(2):
        stat = r_stat[comp] * S_WGT
        for mo in range(2):
            sub = stat[:, mo * 128:(mo + 1) * 128]
            for si in range(4):
                k = ((comp * 2 + mo) * 4 + si) * 2
                put8(k, sub[si * 256:si * 256 + 128])
                put8(k + 1, sub[si * 256 + 128:(si + 1) * 256])
    # zx: [xr xi] (512)
    zx_stat = {0: np.concatenate([w2r, -w2i], axis=0),
               1: np.concatenate([w2i, w2r], axis=0)}
    for comp in range(2):
        stat = zx_stat[comp] * S_WGT
        for mo in range(2):
            sub = stat[:, mo * 128:(mo + 1) * 128]
            for si in range(2):
                k = (16 + (comp * 2 + mo) * 2 + si) * 2
                put8(k, sub[si * 256:si * 256 + 128])
                put8(k + 1, sub[si * 256 + 128:(si + 1) * 256])
    # zh-hr: hr contraction (256); re: r2r, im: r2i
    zh_hr = {0: r2r, 1: r2i}
    for comp in range(2):
        stat = zh_hr[comp] * S_WGT
        for mo in range(2):
            sub = stat[:, mo * 128:(mo + 1) * 128]
            k = (24 + comp * 2 + mo) * 2
            put8(k, sub[0:128])
            put8(k + 1, sub[128:256])

    # --- fp16 stationary -----------------------------------------------
    w16 = np.zeros((128, 40, 128), dtype=np.float32)
    # g3: moving [hr0 nhi0 hr1 nhi1]; G'_re: hr:+r3r, nhi:+r3i
    #                                 G'_im(neg): hr:-r3i, nhi:+r3r
    g3_blocks = {
        0: [r3r[0:128], r3i[0:128], r3r[128:256], r3i[128:256]],
        1: [-r3i[0:128], r3r[0:128], -r3i[128:256], r3r[128:256]],
    }
    for comp in range(2):
        for mo in range(2):
            for j in range(4):
                row = (comp * 2 + mo) * 4 + j
                w16[:, row, :] = g3_blocks[comp][j][:, mo * 128:(mo + 1) * 128]
    # zh-hi: moving [nhi0 nhi1]; z_re: +r2i; z_im: -r2r; scale SC8
    zh_hi = {0: r2i * SC8, 1: -r2r * SC8}
    for comp in range(2):
        for mo in range(2):
            for j in range(2):
                row = 16 + (comp * 2 + mo) * 2 + j
                blk = zh_hi[comp][j * 128:(j + 1) * 128]
                w16[:, row, :] = blk[:, mo * 128:(mo + 1) * 128]
    # x3: moving [xr0 xr1 xi0 xi1]; x3'_re: xr:+w3r, xi:-w3i
    #                               x3'_im: xr:-w3i, xi:-w3r
    x3_blocks = {
        0: [w3r[0:128], w3r[128:256], -w3i[0:128], -w3i[128:256]],
        1: [-w3i[0:128], -w3i[128:256], -w3r[0:128], -w3r[128:256]],
    }
    for comp in range(2):
        for mo in range(2):
            for j in range(4):
                row = 24 + (comp * 2 + mo) * 4 + j
                w16[:, row, :] = x3_blocks[comp][j][:, mo * 128:(mo + 1) * 128]

    # --- biases ---------------------------------------------------------
    def b(n):
        return np.asarray(p[n], np.float32)

    bias = {
        'r_re': b('w1br') - b('w1bi') + b('r1br') - b('r1bi'),
        'r_im': b('w1br') + b('w1bi') + b('r1br') + b('r1bi'),
        'z_re': b('w2br') - b('w2bi') + b('r2br') - b('r2bi'),
        'z_im': b('w2br') + b('w2bi') + b('r2br') + b('r2bi'),
        'g3n_re': b('r3br') - b('r3bi'),
        'g3n_im': -(b('r3br') + b('r3bi')),
        'x3n_re': b('w3br') - b('w3bi'),
        'x3n_im': -(b('w3br') + b('w3bi')),
    }
    bcols = np.zeros((128, 16), dtype=np.float32)
    for gi, g in enumerate(_BIAS_NAMES):
        for mo in range(2):
            bcols[:, gi * 2 + mo] = bias[g][mo * 128:(mo + 1) * 128]

    return {
        "w8": np.clip(w8, -240.0, 240.0).astype(E4M3),
        "w16": w16.astype(np.float16),
        "biases": bcols,
    }


def kernel(**inputs):
    global _CACHED_NC, LAST_RESULT
    if _CACHED_NC is None:
        _CACHED_NC = _build_nc()
    nc = _CACHED_NC

    wmaps = _prep_weights(inputs)

    in_maps = []
    for core in range(N_CORES):
        sl = slice(core * B_LOC, (core + 1) * B_LOC)
        m = dict(wmaps)
        # streams feature-major [4][256, 8192]
        xr = np.asarray(inputs['xr'], np.float32)[sl].T
        xi = np.asarray(inputs['xi'], np.float32)[sl].T
        hr = np.asarray(inputs['hr'], np.float32)[sl].T
        hi = np.asarray(inputs['hi'], np.float32)[sl].T

        # s8 chunk order per tile: xr0 xr1 xi0 xi1 hr0 hr1 hi0 hi1
        s8 = np.empty((128, N_TILES, 8, NB), dtype=np.float32)
        for si, a in enumerate((xr, xi, hr, hi)):
            c = a.reshape(2, 128, N_TILES, NB)
            s8[:, :, 2 * si + 0] = c[0]
            s8[:, :, 2 * si + 1] = c[1]
        s8 = np.clip(s8 * S_MOV, -240.0, 240.0)
        m["s8"] = np.ascontiguousarray(
            s8.astype(E4M3).reshape(128, -1))

        # s16 chunk order per tile: hr0 nhi0 hr1 nhi1 xr0 xr1 xi0 xi1
        s16 = np.empty((128, N_TILES, 8, NB), dtype=np.float32)
        hrc = hr.reshape(2, 128, N_TILES, NB)
        hic = hi.reshape(2, 128, N_TILES, NB)
        xrc = xr.reshape(2, 128, N_TILES, NB)
        xic = xi.reshape(2, 128, N_TILES, NB)
        s16[:, :, 0] = hrc[0]
        s16[:, :, 1] = -hic[0]
        s16[:, :, 2] = hrc[1]
        s16[:, :, 3] = -hic[1]
        s16[:, :, 4] = xrc[0]
        s16[:, :, 5] = xrc[1]
        s16[:, :, 6] = xic[0]
        s16[:, :, 7] = xic[1]
        m["s16"] = np.ascontiguousarray(
            s16.astype(np.float16).reshape(128, -1))
        in_maps.append(m)

    kwargs = {}
    if TRACE:
        import sys, types
        try:
            from trn_agent_boot.trn_boot import _ntff_profile_via_ctypes
            mod = types.ModuleType("antenv.axon_hooks")
            mod._hook = _ntff_profile_via_ctypes('/opt/axon/libaxon_pjrt.so')
            mod.get_axon_ntff_profile_hook = lambda: mod._hook
            mod.set_axon_ntff_profile_hook = (
                lambda h: setattr(mod, "_hook", h))
            sys.modules["antenv.axon_hooks"] = mod
            kwargs["trace"] = True
        except Exception:
            pass

    res = bass_utils.run_bass_kernel_spmd(
        nc, in_maps, core_ids=list(range(N_CORES)), **kwargs)
    LAST_RESULT = res

    out = np.empty((2, B_TOTAL, H), dtype=np.float32)
    for core in range(N_CORES):
        sl = slice(core * B_LOC, (core + 1) * B_LOC)
        o = np.asarray(res.results[core]["outT"], dtype=np.float32)
        out[0, sl] = o[:, 0, :].T
        out[1, sl] = -o[:, 1, :].T
    return out


# revision 21
# speedup vs baseline: 1.0075x; 1.0075x over previous
"""ComplexGRUCell forward on 8 Trainium2 NeuronCores.

v4 strategy (data-parallel, feature-major compute):
  - Shard batch B=65536 across 8 cores (8192 rows each).
  - 34 matmul passes per (tile, mo) [was 36]:
      r gate:  fp8 DoubleRow, contraction 1024, re+im      -> 8 passes
      z gate:  x-part fp8 DR (4), hr-part fp8 DR (2),
               hi-part fp16 (4)                            -> 10 passes
      x3:      fp16, contraction 512, re+im                -> 8 passes
      g3:      fp16, contraction 512, re+im                -> 8 passes
    Mix chosen by offline precision sim: L2 rel ~1.85e-2 < 2e-2 gate.
  - Sign-folded epilogue: the im-halves of x3/g3/out carry a factor -1
    (folded into host-packed weights/biases and un-done on host gather),
    which makes every wide element-wise op a uniform add/sub/mul over
    flat [128, 1024] APs (3D APs halve DVE throughput).
  - Engine balance: Scalar 8 ACTs; DVE wide muls/adds; GpSimd the 4
    cross-term muls of the two complex products.
  - z-gate matmuls mid-scheduled so sigmoid(z) fires before x3 finishes,
    shortening the last-tile drain.

Self-contained: hardcodes B=65536, I=H=256, 8 cores.
"""

import numpy as np
import ml_dtypes

import concourse.bass as bass  # noqa: F401
import concourse.mybir as mybir
import concourse.tile as tile
from concourse import bacc, bass_utils
from concourse.alu_op_type import AluOpType as OP

F32 = mybir.dt.float32
F16 = mybir.dt.float16
F8 = mybir.dt.float8e4
AF = mybir.ActivationFunctionType
PM = mybir.MatmulPerfMode

B_TOTAL = 65536
N_CORES = 8
B_LOC = B_TOTAL // N_CORES  # 8192
H = 256
NB = 512                    # batch columns per tile
N_TILES = B_LOC // NB       # 16

S_MOV = 16.0                # fp8 moving-data scale
S_WGT = 256.0               # fp8 weight scale
SC8 = S_MOV * S_WGT         # fp8-psum scale (4096)
INV_S = 1.0 / SC8
E4M3 = ml_dtypes.float8_e4m3

# bias columns: 8 logical biases x 2 mo
_BIAS_NAMES = ['r_re', 'r_im', 'z_re', 'z_im',
               'g3n_re', 'g3n_im', 'x3n_re', 'x3n_im']

# Module-level knobs for the test harness (grading path leaves them alone).
TRACE = False
LAST_RESULT = None

_CACHED_NC = None


def _build_nc():
    nc = bacc.Bacc("TRN2", target_bir_lowering=False, debug=False,
                   num_devices=N_CORES)

    ins = {}
    # fp8 streams, tile-major: [128, tile, chunk(8), NB]
    # chunk order: xr0 xr1 xi0 xi1 hr0 hr1 hi0 hi1  (DR pairs (0,1)(2,3)..)
    ins["s8"] = nc.dram_tensor("s8", (128, N_TILES * 8 * NB), F8,
                               kind="ExternalInput")
    # fp16 streams, tile-major: [128, tile, chunk(8), NB]
    # chunk order: hr0 nhi0 hr1 nhi1 xr0 xr1 xi0 xi1   (nhi = -hi)
    ins["s16"] = nc.dram_tensor("s16", (128, N_TILES * 8 * NB), F16,
                                kind="ExternalInput")
    # fp8 DR stationary: rows pair up ((2k,2k+1) = one DR pass)
    # r: rows 0:32, zx: 32:48, zh-hr: 48:56
    ins["w8"] = nc.dram_tensor("w8", (128, 56, 128), F8,
                               kind="ExternalInput")
    # fp16 stationary rows: g3: 0:16, zh-hi: 16:24, x3: 24:40
    ins["w16"] = nc.dram_tensor("w16", (128, 40, 128), F16,
                                kind="ExternalInput")
    ins["biases"] = nc.dram_tensor("biases", (128, 16), F32,
                                   kind="ExternalInput")
    # output: [feature, re/nim, batch]  (nim = -im, host negates)
    outT = nc.dram_tensor("outT", (H, 2, B_LOC), F16, kind="ExternalOutput")

    bias_col = {}
    for gi, g in enumerate(_BIAS_NAMES):
        for mo in range(2):
            bias_col[(g, mo)] = gi * 2 + mo

    with tile.TileContext(nc) as tc:
        with (
            tc.tile_pool(name="wpool", bufs=1) as wpool,
            tc.tile_pool(name="m8pool", bufs=3) as m8pool,
            tc.tile_pool(name="m16pool", bufs=3) as m16pool,
            tc.tile_pool(name="spool", bufs=2) as spool,
            tc.tile_pool(name="tpool", bufs=2) as tpool,
            tc.tile_pool(name="opool", bufs=3) as opool,
            tc.tile_pool(name="psum", bufs=1, space="PSUM") as psum,
        ):
            # ---- loads: moving tile 0 first, then weights ----------------
            def load_m8(t_idx):
                t0 = t_idx * (8 * NB)
                t = m8pool.tile([128, 8, NB], F8, name="m8", tag="m8")
                nc.sync.dma_start(t[:], ins["s8"][:, t0:t0 + 8 * NB])
                return t

            def load_m16(t_idx):
                t0 = t_idx * (8 * NB)
                t = m16pool.tile([128, 8 * NB], F16, name="m16", tag="m16")
                nc.sync.dma_start(t[:], ins["s16"][:, t0:t0 + 8 * NB])
                return t

            # All prologue DMAs on the fast sync queue, in just-in-time
            # order: biases, r-weights, m8 t0, zx/zh weights, h-half of
            # m16 t0, g3+zh fp16 weights, x-half of m16 t0, x3 weights.
            w8t = wpool.tile([128, 56, 128], F8, name="w8t", tag="w8t")
            nc.sync.dma_start(w8t[:, 0:32, :], ins["w8"][:, 0:32, :])
            m8_0 = load_m8(0)
            bt = wpool.tile([128, 16], F32, name="bias_t", tag="bias_t")
            nc.sync.dma_start(bt[:], ins["biases"][:])
            nc.sync.dma_start(w8t[:, 32:56, :], ins["w8"][:, 32:56, :])
            m16_0 = m16pool.tile([128, 8 * NB], F16, name="m16", tag="m16")
            nc.sync.dma_start(m16_0[:, 0:4 * NB], ins["s16"][:, 0:4 * NB])
            w16t = wpool.tile([128, 40, 128], F16, name="w16t", tag="w16t")
            nc.sync.dma_start(w16t[:, 0:24, :], ins["w16"][:, 0:24, :])
            nc.sync.dma_start(m16_0[:, 4 * NB:8 * NB], ins["s16"][:, 4 * NB:8 * NB])
            nc.sync.dma_start(w16t[:, 24:40, :], ins["w16"][:, 24:40, :])

            def bias_ap(g, mo):
                c = bias_col[(g, mo)]
                return bt[:, c:c + 1]

            # ---- per batch tile ------------------------------------------
            for t_idx in range(N_TILES):
                if t_idx == 0:
                    m8, m16 = m8_0, m16_0
                else:
                    m8 = load_m8(t_idx)
                    m16 = load_m16(t_idx)

                for mo in range(2):
                    p_r = psum.tile([128, 2 * NB], F32, name=f"pr{mo}",
                                    tag="bkA")
                    p_z = psum.tile([128, 2 * NB], F32, name=f"pz{mo}",
                                    tag="bkB")
                    p_g3 = psum.tile([128, 2 * NB], F32, name=f"pg{mo}",
                                     tag="bkC")
                    p_x3 = psum.tile([128, 2 * NB], F32, name=f"px{mo}",
                                     tag="bkD")

                    # r gate: pairs ((comp*2+mo)*4 + si), si = 4 streams
                    for comp in range(2):
                        dst = p_r[:, comp * NB:(comp + 1) * NB]
                        for si in range(4):
                            k = ((comp * 2 + mo) * 4 + si) * 2
                            nc.tensor.matmul(
                                dst, w8t[:, k:k + 2, :],
                                m8[:, 2 * si:2 * si + 2, :],
                                start=(si == 0), stop=(si == 3),
                                perf_mode=PM.DoubleRow)
                    # zx: pairs 16 + (comp*2+mo)*2 + si, si over xr, xi
                    for comp in range(2):
                        dst = p_z[:, comp * NB:(comp + 1) * NB]
                        for si in range(2):
                            k = (16 + (comp * 2 + mo) * 2 + si) * 2
                            nc.tensor.matmul(
                                dst, w8t[:, k:k + 2, :],
                                m8[:, 2 * si:2 * si + 2, :],
                                start=(si == 0), stop=False,
                                perf_mode=PM.DoubleRow)
                    # zh-hr: pair 24 + comp*2+mo, moving = (hr0, hr1)
                    for comp in range(2):
                        dst = p_z[:, comp * NB:(comp + 1) * NB]
                        k = (24 + comp * 2 + mo) * 2
                        nc.tensor.matmul(
                            dst, w8t[:, k:k + 2, :], m8[:, 4:6, :],
                            start=False, stop=False,
                            perf_mode=PM.DoubleRow)
                    # g3: rows (comp*2+mo)*4 + j, moving chunks j=0..3
                    for comp in range(2):
                        dst = p_g3[:, comp * NB:(comp + 1) * NB]
                        for j in range(4):
                            row = (comp * 2 + mo) * 4 + j
                            nc.tensor.matmul(
                                dst, w16t[:, row, :],
                                m16[:, j * NB:(j + 1) * NB],
                                start=(j == 0), stop=(j == 3))
                    # zh-hi: rows 16 + (comp*2+mo)*2 + j, moving nhi0/1
                    for comp in range(2):
                        dst = p_z[:, comp * NB:(comp + 1) * NB]
                        for j in range(2):
                            row = 16 + (comp * 2 + mo) * 2 + j
                            nc.tensor.matmul(
                                dst, w16t[:, row, :],
                                m16[:, (1 + 2 * j) * NB:(2 + 2 * j) * NB],
                                start=False, stop=(j == 1))
                    # x3: rows 24 + (comp*2+mo)*4 + j, moving chunks 4..7
                    for comp in range(2):
                        dst = p_x3[:, comp * NB:(comp + 1) * NB]
                        for j in range(4):
                            row = 24 + (comp * 2 + mo) * 4 + j
                            nc.tensor.matmul(
                                dst, w16t[:, row, :],
                                m16[:, (4 + j) * NB:(5 + j) * NB],
                                start=(j == 0), stop=(j == 3))

                    # --- epilogue ------------------------------------------
                    sr = spool.tile([128, 2 * NB], F16, name=f"sr{mo}",
                                    tag="sr")
                    gp = spool.tile([128, 2 * NB], F16, name=f"gp{mo}",
                                    tag="gp")
                    sz = spool.tile([128, 2 * NB], F16, name=f"sz{mo}",
                                    tag="sz")
                    P = tpool.tile([128, 2 * NB], F16, name=f"P{mo}", tag="P")
                    qh = tpool.tile([128, NB], F16, name=f"qh{mo}", tag="qh")
                    ql = tpool.tile([128, NB], F16, name=f"ql{mo}", tag="ql")
                    h3 = tpool.tile([128, 2 * NB], F16, name=f"h3{mo}",
                                    tag="h3")
                    ss = tpool.tile([128, 2 * NB], F16, name=f"ss{mo}",
                                    tag="ss")
                    nn = spool.tile([128, 2 * NB], F16, name=f"nn{mo}",
                                    tag="nn")
                    d = tpool.tile([128, 2 * NB], F16, name=f"d{mo}", tag="d")
                    P2 = tpool.tile([128, 2 * NB], F16, name=f"P2{mo}",
                                    tag="P2")
                    q2h = tpool.tile([128, NB], F16, name=f"q2h{mo}",
                                     tag="q2h")
                    q2l = tpool.tile([128, NB], F16, name=f"q2l{mo}",
                                     tag="q2l")
                    tm = tpool.tile([128, 2 * NB], F16, name=f"tm{mo}",
                                    tag="tm")
                    ot = opool.tile([128, 2 * NB], F16, name=f"ot{mo}",
                                    tag="ot")

                    nc.scalar.activation(sr[:, 0:NB], p_r[:, 0:NB],
                                         AF.Sigmoid, bias=bias_ap("r_re", mo),
                                         scale=INV_S)
                    nc.scalar.activation(sr[:, NB:], p_r[:, NB:],
                                         AF.Sigmoid, bias=bias_ap("r_im", mo),
                                         scale=INV_S)
                    # G' = [g3_re + b | -(g3_im + b)] (signs in wts/bias)
                    nc.scalar.activation(gp[:, 0:NB], p_g3[:, 0:NB],
                                         AF.Identity,
                                         bias=bias_ap("g3n_re", mo))
                    nc.scalar.activation(gp[:, NB:], p_g3[:, NB:],
                                         AF.Identity,
                                         bias=bias_ap("g3n_im", mo))
                    nc.scalar.activation(sz[:, 0:NB], p_z[:, 0:NB],
                                         AF.Sigmoid, bias=bias_ap("z_re", mo),
                                         scale=INV_S)
                    nc.scalar.activation(sz[:, NB:], p_z[:, NB:],
                                         AF.Sigmoid, bias=bias_ap("z_im", mo),
                                         scale=INV_S)
                    # h3 = [h3_re | -h3_im]
                    nc.vector.tensor_mul(P[:], sr[:], gp[:])
                    nc.vector.tensor_mul(qh[:], sr[:, 0:NB], gp[:, NB:])
                    nc.vector.tensor_mul(ql[:], sr[:, NB:], gp[:, 0:NB])
                    nc.vector.tensor_add(h3[:, 0:NB], P[:, 0:NB], P[:, NB:])
                    nc.vector.tensor_sub(h3[:, NB:], qh[:], ql[:])
                    # ss = p_x3' + h3   (both carry -im)
                    nc.vector.tensor_add(ss[:], p_x3[:], h3[:])
                    nc.scalar.activation(nn[:, 0:NB], ss[:, 0:NB], AF.Tanh,
                                         bias=bias_ap("x3n_re", mo))
                    nc.scalar.activation(nn[:, NB:], ss[:, NB:], AF.Tanh,
                                         bias=bias_ap("x3n_im", mo))
                    # d' = h' - nn' = [d_re | -d_im]
                    hflat = m16[:, 2 * mo * NB:(2 * mo + 2) * NB]
                    nc.vector.tensor_sub(d[:], hflat, nn[:])
                    # t' = [t_re | -t_im] = z (x) d
                    nc.vector.tensor_mul(P2[:], sz[:], d[:])
                    nc.vector.tensor_mul(q2h[:], sz[:, 0:NB], d[:, NB:])
                    nc.vector.tensor_mul(q2l[:], sz[:, NB:], d[:, 0:NB])
                    nc.vector.tensor_add(tm[:, 0:NB], P2[:, 0:NB],
                                         P2[:, NB:])
                    nc.vector.tensor_sub(tm[:, NB:], q2h[:], q2l[:])
                    # ot' = nn' + t' = [out_re | -out_im]
                    nc.vector.tensor_add(ot[:], nn[:], tm[:])
                    c0 = t_idx * NB
                    ot3 = ot[:].rearrange("p (c n) -> p c n", c=2)
                    nc.sync.dma_start(
                        outT[mo * 128:(mo + 1) * 128, :, c0:c0 + NB],
                        ot3[:])

    nc.compile()
    return nc


def _prep_weights(p):
    """Pack stationary weights + biases (host side, layout only)."""
    def T(name):
        return np.asarray(p[name], np.float32).T  # [in, out]

    w1r, w1i = T('w1Wr'), T('w1Wi')
    r1r, r1i = T('r1Wr'), T('r1Wi')
    w2r, w2i = T('w2Wr'), T('w2Wi')
    r2r, r2i = T('r2Wr'), T('r2Wi')
    w3r, w3i = T('w3Wr'), T('w3Wi')
    r3r, r3i = T('r3Wr'), T('r3Wi')

    # --- fp8 stationary ------------------------------------------------
    w8 = np.zeros((128, 56, 128), dtype=np.float32)

    def put8(row, block):
        w8[:, row, :] = block

    # r gate: stacked contraction [xr xi hr hi] (1024)
    r_stat = {
        0: np.concatenate([w1r, -w1i, r1r, -r1i], axis=0),   # re
        1: np.concatenate([w1i, w1r, r1i, r1r], axis=0),     # im
    }
    for comp in range(2):
        stat = r_stat[comp] * S_WGT
        for mo in range(2):
            sub = stat[:, mo * 128:(mo + 1) * 128]
            for si in range(4):
                k = ((comp * 2 + mo) * 4 + si) * 2
                put8(k, sub[si * 256:si * 256 + 128])
                put8(k + 1, sub[si * 256 + 128:(si + 1) * 256])
    # zx: [xr xi] (512)
    zx_stat = {0: np.concatenate([w2r, -w2i], axis=0),
               1: np.concatenate([w2i, w2r], axis=0)}
    for comp in range(2):
        stat = zx_stat[comp] * S_WGT
        for mo in range(2):
            sub = stat[:, mo * 128:(mo + 1) * 128]
            for si in range(2):
                k = (16 + (comp * 2 + mo) * 2 + si) * 2
                put8(k, sub[si * 256:si * 256 + 128])
                put8(k + 1, sub[si * 256 + 128:(si + 1) * 256])
    # zh-hr: hr contraction (256); re: r2r, im: r2i
    zh_hr = {0: r2r, 1: r2i}
    for comp in range(2):
        stat = zh_hr[comp] * S_WGT
        for mo in range(2):
            sub = stat[:, mo * 128:(mo + 1) * 128]
            k = (24 + comp * 2 + mo) * 2
            put8(k, sub[0:128])
            put8(k + 1, sub[128:256])

    # --- fp16 stationary -----------------------------------------------
    w16 = np.zeros((128, 40, 128), dtype=np.float32)
    # g3: moving [hr0 nhi0 hr1 nhi1]; G'_re: hr:+r3r, nhi:+r3i
    #                                 G'_im(neg): hr:-r3i, nhi:+r3r
    g3_blocks = {
        0: [r3r[0:128], r3i[0:128], r3r[128:256], r3i[128:256]],
        1: [-r3i[0:128], r3r[0:128], -r3i[128:256], r3r[128:256]],
    }
    for comp in range(2):
        for mo in range(2):
            for j in range(4):
                row = (comp * 2 + mo) * 4 + j
                w16[:, row, :] = g3_blocks[comp][j][:, mo * 128:(mo + 1) * 128]
    # zh-hi: moving [nhi0 nhi1]; z_re: +r2i; z_im: -r2r; scale SC8
    zh_hi = {0: r2i * SC8, 1: -r2r * SC8}
    for comp in range(2):
        for mo in range(2):
            for j in range(2):
                row = 16 + (comp * 2 + mo) * 2 + j
                blk = zh_hi[comp][j * 128:(j + 1) * 128]
                w16[:, row, :] = blk[:, mo * 128:(mo + 1) * 128]
    # x3: moving [xr0 xr1 xi0 xi1]; x3'_re: xr:+w3r, xi:-w3i
    #                               x3'_im: xr:-w3i, xi:-w3r
    x3_blocks = {
        0: [w3r[0:128], w3r[128:256], -w3i[0:128], -w3i[128:256]],
        1: [-w3i[0:128], -w3i[128:256], -w3r[0:128], -w3r[128:256]],
    }
    for comp in range(2):
        for mo in range(2):
            for j in range(4):
                row = 24 + (comp * 2 + mo) * 4 + j
                w16[:, row, :] = x3_blocks[comp][j][:, mo * 128:(mo + 1) * 128]

    # --- biases ---------------------------------------------------------
    def b(n):
        return np.asarray(p[n], np.float32)

    bias = {
        'r_re': b('w1br') - b('w1bi') + b('r1br') - b('r1bi'),
        'r_im': b('w1br') + b('w1bi') + b('r1br') + b('r1bi'),
        'z_re': b('w2br') - b('w2bi') + b('r2br') - b('r2bi'),
        'z_im': b('w2br') + b('w2bi') + b('r2br') + b('r2bi'),
        'g3n_re': b('r3br') - b('r3bi'),
        'g3n_im': -(b('r3br') + b('r3bi')),
        'x3n_re': b('w3br') - b('w3bi'),
        'x3n_im': -(b('w3br') + b('w3bi')),
    }
    bcols = np.zeros((128, 16), dtype=np.float32)
    for gi, g in enumerate(_BIAS_NAMES):
        for mo in range(2):
            bcols[:, gi * 2 + mo] = bias[g][mo * 128:(mo + 1) * 128]

    return {
        "w8": np.clip(w8, -240.0, 240.0).astype(E4M3),
        "w16": w16.astype(np.float16),
        "biases": bcols,
    }


def kernel(**inputs):
    global _CACHED_NC, LAST_RESULT
    if _CACHED_NC is None:
        _CACHED_NC = _build_nc()
    nc = _CACHED_NC

    wmaps = _prep_weights(inputs)

    in_maps = []
    for core in range(N_CORES):
        sl = slice(core * B_LOC, (core + 1) * B_LOC)
        m = dict(wmaps)
        # streams feature-major [4][256, 8192]
        xr = np.asarray(inputs['xr'], np.float32)[sl].T
        xi = np.asarray(inputs['xi'], np.float32)[sl].T
        hr = np.asarray(inputs['hr'], np.float32)[sl].T
        hi = np.asarray(inputs['hi'], np.float32)[sl].T

        # s8 chunk order per tile: xr0 xr1 xi0 xi1 hr0 hr1 hi0 hi1
        s8 = np.empty((128, N_TILES, 8, NB), dtype=np.float32)
        for si, a in enumerate((xr, xi, hr, hi)):
            c = a.reshape(2, 128, N_TILES, NB)
            s8[:, :, 2 * si + 0] = c[0]
            s8[:, :, 2 * si + 1] = c[1]
        s8 = np.clip(s8 * S_MOV, -240.0, 240.0)
        m["s8"] = np.ascontiguousarray(
            s8.astype(E4M3).reshape(128, -1))

        # s16 chunk order per tile: hr0 nhi0 hr1 nhi1 xr0 xr1 xi0 xi1
        s16 = np.empty((128, N_TILES, 8, NB), dtype=np.float32)
        hrc = hr.reshape(2, 128, N_TILES, NB)
        hic = hi.reshape(2, 128, N_TILES, NB)
        xrc = xr.reshape(2, 128, N_TILES, NB)
        xic = xi.reshape(2, 128, N_TILES, NB)
        s16[:, :, 0] = hrc[0]
        s16[:, :, 1] = -hic[0]
        s16[:, :, 2] = hrc[1]
        s16[:, :, 3] = -hic[1]
        s16[:, :, 4] = xrc[0]
        s16[:, :, 5] = xrc[1]
        s16[:, :, 6] = xic[0]
        s16[:, :, 7] = xic[1]
        m["s16"] = np.ascontiguousarray(
            s16.astype(np.float16).reshape(128, -1))
        in_maps.append(m)

    kwargs = {}
    if TRACE:
        import sys, types
        try:
            from trn_agent_boot.trn_boot import _ntff_profile_via_ctypes
            mod = types.ModuleType("antenv.axon_hooks")
            mod._hook = _ntff_profile_via_ctypes('/opt/axon/libaxon_pjrt.so')
            mod.get_axon_ntff_profile_hook = lambda: mod._hook
            mod.set_axon_ntff_profile_hook = (
                lambda h: setattr(mod, "_hook", h))
            sys.modules["antenv.axon_hooks"] = mod
            kwargs["trace"] = True
        except Exception:
            pass

    res = bass_utils.run_bass_kernel_spmd(
        nc, in_maps, core_ids=list(range(N_CORES)), **kwargs)
    LAST_RESULT = res

    out = np.empty((2, B_TOTAL, H), dtype=np.float32)
    for core in range(N_CORES):
        sl = slice(core * B_LOC, (core + 1) * B_LOC)
        o = np.asarray(res.results[core]["outT"], dtype=np.float32)
        out[0, sl] = o[:, 0, :].T
        out[1, sl] = -o[:, 1, :].T
    return out
